# revision 50
# baseline (speedup 1.0000x reference)
"""Trainium2 Bass kernel for HeatmapMaxDetBlock (argmax + local refinement).

Computes, for x[B, C, H, W]:
    scores = max over (H*W); idx = argmax; px = idx % W, py = idx // W (masked
    by score > 0); quarter-pixel refinement by sign of neighbor differences.
Returns [B, C, 3] = (px, py, scores).

Strategy (pure data parallel over 8 NeuronCores, batch-sharded):
  phase 1: stream the whole shard through SBUF once; one DVE reduce_max per
           [128, SEGW] tile gives per-(row, segment) maxima.
  phase 2: tiny ops — PE-transpose the maxima, per-row max + winning segment,
           one indirect-DMA window gather per row group (window includes +-W
           margins), max_index for the exact in-segment position, then
           refinement WITHOUT a second gather: the +-1 / +-W neighbor
           differences are computed over the whole window (on Pool) and
           selected at the peak with fused one-hot scalar_tensor_tensor
           sum-accumulate ops.  px/py need no division chain since
           SEGW = 4*W: py = sb/W + sum_k(ii >= k*W), px = idxm - W*py.
           The last stream DMAs taper (MD 4 -> 2) so the final exposed
           reduce is half-size.
"""

import sys
from contextlib import ExitStack
from dataclasses import dataclass

import numpy as np

for _p in ("/opt/trn_rl_repo",):
    if _p not in sys.path:
        sys.path.insert(0, _p)

import concourse.bass as bass  # noqa: E402
import concourse.tile as tile  # noqa: E402
from concourse import bacc, mybir  # noqa: E402
from concourse.masks import make_identity  # noqa: E402

F32 = mybir.dt.float32
U32 = mybir.dt.uint32
AX = mybir.AxisListType
OP = mybir.AluOpType


@dataclass(frozen=True)
class Cfg:
    B: int = 64
    C: int = 17
    H: int = 256
    W: int = 192
    ncores: int = 8
    P: int = 128
    NSEG: int = 64
    MD: int = 4  # tile-columns merged per DMA
    FRONT: int = 256
    REAR: int = 512

    @property
    def BP(self):  # batches per core
        return self.B // self.ncores

    @property
    def R(self):  # heatmap rows per core
        return self.BP * self.C

    @property
    def HWm(self):
        return self.H * self.W

    @property
    def SEGW(self):
        return self.HWm // self.NSEG

    @property
    def RPT(self):  # rows per tile
        return self.P // self.NSEG

    @property
    def NT(self):  # tiles per core
        return self.R // self.RPT

    @property
    def MARG(self):
        return self.W + 2

    @property
    def WINW(self):
        return self.SEGW + 2 * self.MARG

    @property
    def NBW(self):
        return 2 * self.W + 1

    @property
    def SHN(self):
        return self.R * self.HWm

    @property
    def NPAD(self):
        return self.FRONT + self.SHN + self.REAR


CFG = Cfg()


def build_program(cfg: Cfg):
    c = cfg
    assert c.P % c.NSEG == 0 and c.R % c.RPT == 0 and c.HWm % c.NSEG == 0
    assert c.R <= c.P or c.R - c.P in range(0, 17), (
        "group B must fit in one 16-partition pad"
    )
    assert c.FRONT >= c.MARG and c.REAR >= c.MARG
    assert 8 <= c.SEGW <= 16384
    assert c.NT % c.MD == 0 and c.NT <= c.P
    GA = min(c.P, c.R)
    assert GA % c.RPT == 0

    nc = bacc.Bacc(
        "TRN2", target_bir_lowering=False, debug=False, num_devices=c.ncores
    )
    xh = nc.dram_tensor("x", [c.NPAD], F32, kind="ExternalInput").ap()
    rbh = nc.dram_tensor("rowbase", [c.NT, c.RPT], F32, kind="ExternalInput").ap()
    irh = nc.dram_tensor("iotarev", [c.NT, c.P], F32, kind="ExternalInput").ap()
    iofh = nc.dram_tensor("iota768", [c.P, c.SEGW], F32, kind="ExternalInput").ap()
    oh = nc.dram_tensor("out", [c.R, 3], F32, kind="ExternalOutput").ap()

    with ExitStack() as ctx:
        tc = ctx.enter_context(tile.TileContext(nc))
        xpool = ctx.enter_context(tc.tile_pool(name="xp", bufs=3))
        sp = ctx.enter_context(tc.tile_pool(name="sp", bufs=1))
        pp = ctx.enter_context(tc.tile_pool(name="pp", bufs=1, space="PSUM"))

        # ---- phase 1: per-(row, segment) maxima ------------------------------
        # DMA tile g: [P, MD*SEGW]; partition p = RPT-row j * NSEG + seg s;
        # free = MD tile-columns (row-groups) of SEGW. One reduce per DMA
        # yields MD columns of M.
        M = sp.tile([c.P, c.P], F32, tag="M")
        nc.vector.memset(M[:], 0.0)
        ident = sp.tile([c.P, c.P], F32, tag="ident")
        make_identity(nc, ident[:])
        irt = sp.tile([c.NT, c.P], F32, tag="irt")
        nc.sync.dma_start(out=irt[:], in_=irh[:])
        iof = sp.tile([c.P, c.SEGW], F32, tag="iof")
        nc.scalar.dma_start(out=iof[:], in_=iofh[:])
        rbt = sp.tile([c.NT, c.RPT], F32, tag="rbt")
        nc.sync.dma_start(out=rbt[:], in_=rbh[:])
        MT = sp.tile([c.P, c.P], F32, tag="MT")
        scores = sp.tile([c.NT, c.RPT], F32, tag="scores")
        mk16 = sp.tile([c.NT, c.P], F32, tag="mk16")
        srev = sp.tile([c.NT, c.RPT], F32, tag="srev")
        sb = sp.tile([c.NT, c.RPT], F32, tag="sb")
        w0 = sp.tile([c.NT, c.RPT], F32, tag="w0")
        P4 = sp.tile([c.NT, c.RPT * 3], F32, tag="P4")
        P43 = P4[:].rearrange("p (j e) -> p j e", e=3)

        def select_block(lo, hi, tagp):
            # winner segment + window start for tile-columns [lo, hi) --
            # all ops on same-base slices so SB base partitions match.
            n = hi - lo
            mtp = pp.tile([n, c.P], F32, tag=f"mtp{tagp}", name=f"mtp{tagp}")
            nc.tensor.transpose(out=mtp[:], in_=M[:, lo:hi], identity=ident[:])
            nc.vector.tensor_copy(out=MT[lo:hi], in_=mtp[:])
            MT3s = MT[lo:hi].rearrange("p (j s) -> p j s", j=c.RPT)
            mk3s = mk16[lo:hi].rearrange("p (j s) -> p j s", j=c.RPT)
            nc.vector.reduce_max(out=scores[lo:hi], in_=MT3s, axis=AX.X)
            nc.vector.tensor_tensor(
                out=mk3s,
                in0=MT3s,
                in1=scores[lo:hi, :, None].to_broadcast([n, c.RPT, c.NSEG]),
                op=OP.is_equal,
            )
            nc.vector.tensor_tensor(
                out=mk3s,
                in0=mk3s,
                in1=irt[lo:hi].rearrange("p (j s) -> p j s", j=c.RPT),
                op=OP.mult,
            )
            nc.vector.reduce_max(out=srev[lo:hi], in_=mk3s, axis=AX.X)
            nc.vector.tensor_scalar(
                out=sb[lo:hi],
                in0=srev[lo:hi],
                scalar1=-float(c.SEGW),
                scalar2=float((c.NSEG - 1) * c.SEGW),
                op0=OP.mult,
                op1=OP.add,
            )
            nc.vector.tensor_tensor(
                out=w0[lo:hi], in0=sb[lo:hi], in1=rbt[lo:hi], op=OP.add
            )
            for e, srcv in enumerate((w0, scores, sb)):
                nc.vector.tensor_copy(
                    out=P43[lo:hi, :, e : e + 1], in_=srcv[lo:hi, :, None]
                )

        mds = [4] * 16 + [2, 2]  # tapered so the last reduce is small
        assert sum(mds) == c.NT
        col = 0
        RA = None
        for g, md in enumerate(mds):
            xt = xpool.tile([c.P, md * c.SEGW], F32, tag=f"xt{md}")
            off = c.FRONT + col * c.RPT * c.HWm
            src = bass.AP(
                xh.tensor,
                off,
                [
                    [c.HWm, c.RPT],
                    [c.SEGW, c.NSEG],
                    [c.RPT * c.HWm, md],
                    [1, c.SEGW],
                ],
            )
            eng = nc.sync if g % 2 == 0 else nc.scalar
            eng.dma_start(
                out=xt[:].rearrange("p (m u) -> p m u", m=md), in_=src
            )
            nc.vector.reduce_max(
                out=M[:, col : col + md],
                in_=xt[:].rearrange("p (m u) -> p m u", m=md),
                axis=AX.X,
            )
            col += md
            if col == GA // c.RPT:
                # group A's tile-columns are all reduced: run its winner
                # select while group B's tiles still stream, so A's window
                # gather is already in flight at stream end.
                select_block(0, GA // c.RPT, "a")
            elif col > GA // c.RPT and RA is None:
                # relayout AFTER the next chunk's DMA issue so sync's queue
                # never blocks a pending stream issue on the pack semaphore
                RA = sp.tile([GA, 3], F32, tag="RA")
                nc.sync.dma_start(out=RA[:], in_=P43[0 : GA // c.RPT])

        # ---- group B winner select (after the last reduces) ------------------
        nta = GA // c.RPT  # tile-columns covered by group A
        select_block(nta, c.NT, "b")
        RB = sp.tile([16, 3], F32, tag="RB")
        nc.vector.memset(RB[:], 0.0)
        nc.sync.dma_start(out=RB[0 : c.R - c.P], in_=P43[nta : c.NT])

        # ---- phase 2b/c per row group ---------------------------------------
        def gather_pre(Rt, gp, tagp):
            w0u = sp.tile([gp, 1], U32, tag=f"w0u{tagp}")
            nc.vector.tensor_copy(out=w0u[:], in_=Rt[:, 0:1])
            win = sp.tile([gp, c.WINW], F32, tag=f"win{tagp}")
            nc.gpsimd.indirect_dma_start(
                out=win[:],
                out_offset=None,
                in_=xh[:, None],
                in_offset=bass.IndirectOffsetOnAxis(ap=w0u[:, 0:1], axis=0),
            )
            return win

        def diffs_pre(win, gp, tagp):
            M0 = c.MARG
            diff = sp.tile([gp, 2 * c.SEGW], F32, tag=f"df{tagp}")
            nc.gpsimd.tensor_tensor(
                out=diff[:, 0 : c.SEGW],
                in0=win[:, M0 + 1 : M0 + 1 + c.SEGW],
                in1=win[:, M0 - 1 : M0 - 1 + c.SEGW],
                op=OP.subtract,
            )
            nc.gpsimd.tensor_tensor(
                out=diff[:, c.SEGW : 2 * c.SEGW],
                in0=win[:, M0 + c.W : M0 + c.W + c.SEGW],
                in1=win[:, M0 - c.W : M0 - c.W + c.SEGW],
                op=OP.subtract,
            )
            return diff

        def group_idx(Rt, win, gp, tagp):
            m8 = sp.tile([gp, 8], F32, tag=f"m8{tagp}")
            nc.vector.tensor_copy(out=m8[:], in_=Rt[:, 1:2].to_broadcast([gp, 8]))
            mi = sp.tile([gp, 8], U32, tag=f"mi{tagp}")
            nc.vector.max_index(
                mi[:], m8[:], win[:, c.MARG : c.MARG + c.SEGW]
            )
            ii = sp.tile([gp, 1], F32, tag=f"ii{tagp}")
            nc.vector.tensor_copy(out=ii[:], in_=mi[:, 0:1])
            return ii

        def group_sel(ii, diff, gp, tagp):
            # one-hot select of the +-1 / +-W differences at the peak
            scrD = sp.tile([gp, c.SEGW], F32, tag=f"scrD{tagp}")
            D = sp.tile([gp, 2], F32, tag=f"D{tagp}")
            nc.vector.scalar_tensor_tensor(
                out=scrD[:], in0=iof[0:gp], scalar=ii[:],
                in1=diff[:, 0 : c.SEGW],
                op0=OP.is_equal, op1=OP.mult, accum_out=D[:, 0:1],
            )
            nc.vector.scalar_tensor_tensor(
                out=scrD[:], in0=iof[0:gp], scalar=ii[:],
                in1=diff[:, c.SEGW : 2 * c.SEGW],
                op0=OP.is_equal, op1=OP.mult, accum_out=D[:, 1:2],
            )
            return D

        def group_math(Rt, ii, D, gp, tagp):

            # final math: SEGW = 4*W, so py = sb/W + sum_k (ii >= k*W) and
            # px = idxm - W*py -- all exact integer f32, no casts needed
            O = sp.tile([gp, 3], F32, tag=f"O{tagp}")
            idxm = sp.tile([gp, 1], F32, tag=f"idxm{tagp}")
            nc.vector.tensor_tensor(out=idxm[:], in0=Rt[:, 2:3], in1=ii[:], op=OP.add)
            t1 = sp.tile([gp, 1], F32, tag=f"t1{tagp}")
            t2 = sp.tile([gp, 1], F32, tag=f"t2{tagp}")
            lo = sp.tile([gp, 1], F32, tag=f"lo{tagp}")
            nc.vector.tensor_scalar(
                out=t1[:], in0=ii[:], scalar1=float(c.W), scalar2=None, op0=OP.is_ge
            )
            nc.vector.tensor_scalar(
                out=t2[:], in0=ii[:], scalar1=float(2 * c.W), scalar2=None,
                op0=OP.is_ge,
            )
            nc.vector.tensor_scalar(
                out=lo[:], in0=ii[:], scalar1=float(3 * c.W), scalar2=None,
                op0=OP.is_ge,
            )
            nc.vector.tensor_tensor(out=t1[:], in0=t1[:], in1=t2[:], op=OP.add)
            nc.vector.tensor_tensor(out=t1[:], in0=t1[:], in1=lo[:], op=OP.add)
            nc.vector.tensor_scalar(
                out=t2[:], in0=Rt[:, 2:3], scalar1=1.0 / c.W, scalar2=None,
                op0=OP.mult,
            )
            nc.vector.tensor_tensor(out=O[:, 1:2], in0=t2[:], in1=t1[:], op=OP.add)
            nc.vector.scalar_tensor_tensor(
                out=O[:, 0:1], in0=O[:, 1:2], scalar=-float(c.W), in1=idxm[:],
                op0=OP.mult, op1=OP.add,
            )
            mk = sp.tile([gp, 1], F32, tag=f"mk{tagp}")
            nc.vector.tensor_scalar(
                out=mk[:], in0=Rt[:, 1:2], scalar1=0.0, scalar2=None, op0=OP.is_gt
            )
            nc.vector.tensor_tensor(
                out=O[:, 0:2], in0=O[:, 0:2],
                in1=mk[:].to_broadcast([gp, 2]), op=OP.mult,
            )
            # interior = (0 < px < W-1) & (0 < py < H-1)
            hi = sp.tile([gp, 2], F32, tag=f"hi{tagp}")
            nc.vector.memset(hi[:, 0:1], float(c.W - 1))
            nc.vector.memset(hi[:, 1:2], float(c.H - 1))
            ilo = sp.tile([gp, 2], F32, tag=f"ilo{tagp}")
            nc.vector.tensor_scalar(
                out=ilo[:], in0=O[:, 0:2], scalar1=0.0, scalar2=None, op0=OP.is_gt
            )
            ihi = sp.tile([gp, 2], F32, tag=f"ihi{tagp}")
            nc.vector.tensor_tensor(out=ihi[:], in0=O[:, 0:2], in1=hi[:], op=OP.is_lt)
            nc.vector.tensor_tensor(out=ilo[:], in0=ilo[:], in1=ihi[:], op=OP.mult)
            intr = sp.tile([gp, 1], F32, tag=f"intr{tagp}")
            nc.vector.tensor_reduce(out=intr[:], in_=ilo[:], axis=AX.X, op=OP.min)

            # dx, dy = 0.25 * sign(D) * interior
            DG = sp.tile([gp, 2], F32, tag=f"DG{tagp}")
            DL = sp.tile([gp, 2], F32, tag=f"DL{tagp}")
            nc.vector.tensor_scalar(
                out=DG[:], in0=D[:], scalar1=0.0, scalar2=0.25,
                op0=OP.is_gt, op1=OP.mult,
            )
            nc.vector.tensor_scalar(
                out=DL[:], in0=D[:], scalar1=0.0, scalar2=0.25,
                op0=OP.is_lt, op1=OP.mult,
            )
            nc.vector.tensor_tensor(out=DG[:], in0=DG[:], in1=DL[:], op=OP.subtract)
            nc.vector.tensor_tensor(
                out=DG[:], in0=DG[:], in1=intr[:].to_broadcast([gp, 2]), op=OP.mult
            )
            nc.vector.tensor_tensor(out=O[:, 0:2], in0=O[:, 0:2], in1=DG[:], op=OP.add)
            nc.vector.tensor_copy(out=O[:, 2:3], in_=Rt[:, 1:2])
            return O

        winA = gather_pre(RA[:], GA, "a")
        winB = gather_pre(RB[:], 16, "b")
        diffA = diffs_pre(winA, GA, "a")
        diffB = diffs_pre(winB, 16, "b")
        iiA = group_idx(RA[:], winA, GA, "a")
        iiB = group_idx(RB[:], winB, 16, "b")
        DA = group_sel(iiA, diffA, GA, "a")
        DB = group_sel(iiB, diffB, 16, "b")
        OA = group_math(RA[:], iiA, DA, GA, "a")
        nc.sync.dma_start(out=oh[0:GA], in_=OA[:])
        OB = group_math(RB[:], iiB, DB, 16, "b")
        nc.scalar.dma_start(out=oh[c.P : c.R], in_=OB[0 : c.R - c.P])

    nc.compile()
    return nc


def host_constants(cfg: Cfg):
    c = cfg
    r = np.arange(c.R, dtype=np.float64)
    rowbase = (c.FRONT + r * c.HWm - c.MARG).astype(np.float32).reshape(c.NT, c.RPT)
    s = np.arange(c.NSEG, dtype=np.float64)
    row = np.tile((c.NSEG - 1 - s), c.RPT).astype(np.float32)  # [P]
    iotarev = np.tile(row, (c.NT, 1)).astype(np.float32)
    iota768 = np.tile(np.arange(c.SEGW, dtype=np.float32), (c.P, 1))
    return rowbase, iotarev, iota768


def shard_inputs(cfg: Cfg, x: np.ndarray):
    c = cfg
    rowbase, iotarev, iota768 = host_constants(c)
    in_maps = []
    for k in range(c.ncores):
        shard = np.ascontiguousarray(
            x[k * c.BP : (k + 1) * c.BP], dtype=np.float32
        ).reshape(-1)
        xp = np.zeros(c.NPAD, np.float32)
        xp[c.FRONT : c.FRONT + c.SHN] = shard
        in_maps.append(
            {"x": xp, "rowbase": rowbase, "iotarev": iotarev, "iota768": iota768}
        )
    return in_maps


def assemble_out(cfg: Cfg, per_core_outs):
    c = cfg
    outs = [o.reshape(c.BP, c.C, 3).astype(np.float32) for o in per_core_outs]
    return np.concatenate(outs, axis=0)


_PROGRAM = None


def _program():
    global _PROGRAM
    if _PROGRAM is None:
        _PROGRAM = build_program(CFG)
    return _PROGRAM


def kernel(x: np.ndarray) -> np.ndarray:
    from concourse.bass_utils import run_bass_kernel_spmd

    c = CFG
    assert x.shape == (c.B, c.C, c.H, c.W), x.shape
    nc = _program()
    in_maps = shard_inputs(c, np.asarray(x))
    res = run_bass_kernel_spmd(nc, in_maps, core_ids=list(range(c.ncores)))
    return assemble_out(c, [res.results[k]["out"] for k in range(c.ncores)])

